# revision 1
# baseline (speedup 1.0000x reference)
"""Trainium2 Bass kernel for nn_DA_conv: per-sample dynamic depthwise 3x3 conv
(+LeakyReLU) followed by a 1x1 pointwise conv, with the 3x3 kernels produced by
a small per-sample MLP.

Strategy (8 NeuronCores, pure batch data-parallel, 2 samples per core):
  - SBUF layout: partition p = (sample s = p//64, channel c = p%64); the whole
    2-sample feature map lives resident in SBUF with zero-padded borders so
    every conv tap is a plain strided access-pattern read.
  - The kernel-generating MLP runs on the TensorEngine (tiny matmuls).
  - Depthwise 3x3 conv = 9 PSUM-accumulating diagonal matmuls per output tile.
    Diagonal 32x32 weight blocks + 32x32 TensorE array tiling (16 independent
    sub-tiles addressed via tile_position) recover the concurrency a depthwise
    contraction otherwise wastes on the 128x128 array.
  - LeakyReLU is fused into the PSUM->SBUF evacuation on the Scalar engine.
  - 1x1 conv = dense 32x32-tiled matmuls (contraction over channels), bias add
    fused into the PSUM->SBUF evacuation on the Vector engine.
  - Matmuls run in float32r (full-rate fp32 path; fp32 proper is 4x slower).
  - Emission is software-pipelined over half-blocks (depthwise of half m, then
    1x1 of half m-1) so PSUM evacuations overlap the next depthwise group.
"""

import os
import sys

sys.path.insert(0, "/opt/trn_rl_repo")

from contextlib import ExitStack

import numpy as np

import concourse.bacc as bacc
import concourse.bass as bass
import concourse.mybir as mybir
import concourse.tile as tile

S = 2            # samples per core
C = 64           # channels
H = W = 128      # spatial
KK = 3           # conv kernel size
NCORES = 8
RS = 132         # padded row stride in elements (16B-aligned: 132*4 = 528)
RP = H + 2       # padded row count (top/bottom halo)
XFREE = RP * RS  # padded image elements per partition
BR = 8           # image rows per block
NBLK = H // BR   # 16 blocks
HPX = (BR // 2) * W  # 512 pixels per half-block = one PSUM bank

f32 = mybir.dt.float32
f32r = mybir.dt.float32r
bf16 = mybir.dt.bfloat16
i32 = mybir.dt.int32

# x dtype for the depthwise matmuls. "f32r" keeps full fp32 DMA traffic;
# "bf16" halves the input DMA at a small accuracy cost.
X_MODE = os.environ.get("DA_CONV_X_MODE", "bf16")

LRELU = mybir.ActivationFunctionType.Lrelu
LRELU_MODE = os.environ.get("DA_CONV_LRELU", "prelu")
TAPS = [(di, dj) for di in range(KK) for dj in range(KK)]  # t = di*3 + dj


def build_program(x_mode: str = X_MODE) -> bass.Bass:
    # NOTE: fp32r matmuls cannot use TensorE column tiling on this toolchain
    # (s3d3_mm_valid_dst_partition), so the tiled conv stages must be bf16.
    xdt = bf16

    nc = bacc.Bacc("TRN2", target_bir_lowering=False, debug=False)

    x_d = nc.dram_tensor("x", [S * C, H * W], xdt, kind="ExternalInput").ap()
    dt_d = nc.dram_tensor("dT", [C, S], f32, kind="ExternalInput").ap()
    wk1_d = nc.dram_tensor("wk1t", [C, C], f32, kind="ExternalInput").ap()
    # Wk2 transposed + tap-major + duplicated over samples:
    # wk2td[j, t*128 + s*64 + c] = Wk2[c*9 + t, j]
    wk2_d = nc.dram_tensor("wk2td", [C, KK * KK * 2 * C], f32, kind="ExternalInput").ap()
    wct2_d = nc.dram_tensor("wct2", [2 * C, C], bf16, kind="ExternalInput").ap()
    bc_d = nc.dram_tensor("bc2", [2 * C, 1], f32, kind="ExternalInput").ap()
    out_d = nc.dram_tensor("out", [S * C, H * W], f32, kind="ExternalOutput").ap()

    with tile.TileContext(nc) as tc, ExitStack() as ctx:
        _body(ctx, tc, x_d, dt_d, wk1_d, wk2_d, wct2_d, bc_d, out_d, xdt)
    nc.compile()
    return nc


def _body(ctx, tc, x_d, dt_d, wk1_d, wk2_d, wct2_d, bc_d, out_d, xdt):
    nc = tc.nc
    const = ctx.enter_context(tc.tile_pool(name="const", bufs=1))
    xpool = ctx.enter_context(tc.tile_pool(name="xs", bufs=1))
    dwlp = ctx.enter_context(tc.tile_pool(name="dwl", bufs=4))
    abtp = ctx.enter_context(tc.tile_pool(name="abt", bufs=4))
    o2p = ctx.enter_context(tc.tile_pool(name="o2", bufs=NBLK // 2))
    pdw = ctx.enter_context(tc.tile_pool(name="pdw", bufs=2, space="PSUM"))
    po2 = ctx.enter_context(tc.tile_pool(name="po2", bufs=2, space="PSUM"))

    # ---------------- small-weight loads ----------------
    wk1t = const.tile([C, C], f32)
    nc.sync.dma_start(wk1t[:, :], wk1_d)
    wk2td = const.tile([C, KK * KK * 2 * C], f32)
    nc.sync.dma_start(wk2td[:, :], wk2_d)
    dts = const.tile([C, S], f32)
    nc.sync.dma_start(dts[:, :], dt_d)
    wct2 = const.tile([2 * C, C], bf16)
    nc.sync.dma_start(wct2[:, :], wct2_d)
    bc2 = const.tile([2 * C, 1], f32)
    nc.sync.dma_start(bc2[:, :], bc_d)

    # ---------------- kernel-generating MLP ----------------
    # hid[j, s] = lrelu(sum_i Wk1[j, i] d[s, i])  via lhsT = Wk1.T
    hid_ps = po2.tile([C, S], f32, tag="oe")
    nc.tensor.matmul(
        hid_ps[:, :], lhsT=wk1t[:, :], rhs=dts[:, :], start=True, stop=True,
    )
    hid_sb = const.tile([C, S], f32)
    if LRELU_MODE == "prelu":
        nc.scalar.activation(hid_sb[:, :], hid_ps[:, :],
                             mybir.ActivationFunctionType.Prelu, alpha=0.1)
    else:
        hid_ab = const.tile([C, S], f32)
        nc.scalar.activation(hid_ab[:, :], hid_ps[:, :],
                             mybir.ActivationFunctionType.Abs, scale=0.45)
        nc.vector.scalar_tensor_tensor(
            hid_sb[:, :], hid_ps[:, :], 0.55, hid_ab[:, :],
            op0=mybir.AluOpType.mult, op1=mybir.AluOpType.add,
        )

    # kern tap columns: kcols[s*64+c, t] = kern[s, c*9+t]
    kcols = const.tile([2 * C, KK * KK], f32)
    for t in range(KK * KK):
        kp = po2.tile([2 * C, S], f32, tag="oe")
        nc.tensor.matmul(
            kp[:, :],
            lhsT=wk2td[:, t * 128 : (t + 1) * 128],
            rhs=hid_sb[:, :],
            start=True, stop=True,
        )
        # partition p wants free column s = p//64 of kp (partition-aligned copies)
        nc.vector.tensor_copy(kcols[0:C, t : t + 1], kp[0:C, 0:1])
        nc.vector.tensor_copy(kcols[C : 2 * C, t : t + 1], kp[C : 2 * C, 1:2])

    # identity -> per-tap diagonal weight matrices diag[:, t*128:(t+1)*128]
    id_i = const.tile([128, 128], i32)
    nc.gpsimd.iota(id_i[:, :], pattern=[[1, 128]], base=0, channel_multiplier=-1)
    idf = const.tile([128, 128], f32)
    nc.vector.tensor_scalar(idf[:, :], id_i[:, :], 0, None, mybir.AluOpType.is_equal)
    diag = const.tile([128, KK * KK * 128], xdt)
    for t in range(KK * KK):
        nc.vector.tensor_scalar_mul(
            diag[:, t * 128 : (t + 1) * 128], idf[:, :], kcols[:, t : t + 1]
        )

    # ---------------- resident padded feature map ----------------
    xs = xpool.tile([128, XFREE], xdt)
    # top halo row + row-1 left pad (contiguous), bottom halo row, and the
    # pad columns: right-pad of row r is contiguous with left-pad of row r+1,
    # so one strided memset covers all interior pad columns.
    nc.vector.memset(xs[:, 0 : RS + 1], 0.0)
    nc.vector.memset(xs[:, (RP - 1) * RS : RP * RS], 0.0)
    pads = xs[:, W + 1 : W + 1 + (H + 1) * RS].rearrange("p (r w) -> p r w", w=RS)
    nc.vector.memset(pads[:, :, 0:4], 0.0)
    # image rows in 16 chunks so compute can start early
    for k in range(NBLK):
        src = x_d[:, k * BR * W : (k + 1) * BR * W].rearrange(
            "p (r w) -> p r w", w=W
        )
        o = (k * BR + 1) * RS + 1
        dst = xs[:, o : o + BR * RS].rearrange("p (r w) -> p r w", w=RS)[:, :, 0:W]
        nc.sync.dma_start(dst, src)

    # ---------------- main loop ----------------
    # 64x64 TensorE tiling: 4 concurrent positions. Each PSUM bank has exactly
    # one row-tile writer (HW constraint): P_A <- row tile 0 (sample A
    # channels), P_B <- row tile 1; column groups select the pixel half (E =
    # rows 8k..8k+3, O = rows 8k+4..8k+7) within the bank.
    xrows = xs[:, :].rearrange("p (r w) -> p r w", w=RS)

    def lrelu_evac(D, P):
        if LRELU_MODE == "prelu":
            nc.scalar.activation(D[:, :], P[:, :],
                                 mybir.ActivationFunctionType.Prelu, alpha=0.1)
        else:
            # lrelu(x) = 0.55x + 0.45|x| ; Abs on ScalarE, fused MAC on VectorE
            ab = abtp.tile([128, HPX], f32, tag="abt")
            nc.scalar.activation(ab[:, :], P[:, :],
                                 mybir.ActivationFunctionType.Abs, scale=0.45)
            nc.vector.scalar_tensor_tensor(
                D[:, :], P[:, :], 0.55, ab[:, :],
                op0=mybir.AluOpType.mult, op1=mybir.AluOpType.add,
            )

    def dw_stage(k):
        r0e = BR * k
        r0o = BR * k + BR // 2
        PA = pdw.tile([128, HPX], f32, tag="pa")
        PB = pdw.tile([128, HPX], f32, tag="pb")
        for t, (di, dj) in enumerate(TAPS):
            wE = xrows[:, r0e + di : r0e + di + 4, dj : dj + W]
            wO = xrows[:, r0o + di : r0o + di + 4, dj : dj + W]
            la = diag[0:C, t * 128 : t * 128 + C]
            lb = diag[C : 2 * C, t * 128 + C : t * 128 + 2 * C]
            for cg, win in ((0, wE), (C, wO)):
                nc.tensor.matmul(
                    PA[cg : cg + C, :], lhsT=la, rhs=win[0:C, :, :],
                    start=(t == 0), stop=(t == KK * KK - 1),
                    tile_position=(0, cg), skip_group_check=True,
                )
                nc.tensor.matmul(
                    PB[cg : cg + C, :], lhsT=lb, rhs=win[C : 2 * C, :, :],
                    start=(t == 0), stop=(t == KK * KK - 1),
                    tile_position=(C, cg), skip_group_check=True,
                )
        DA = dwlp.tile([128, HPX], bf16, tag="da")
        DB = dwlp.tile([128, HPX], bf16, tag="db")
        lrelu_evac(DA, PA)
        lrelu_evac(DB, PB)
        return k, DA, DB

    def conv1x1_stage(k, DA, DB):
        OE = po2.tile([128, HPX], f32, tag="oe")
        OO = po2.tile([128, HPX], f32, tag="oo")
        # E outputs via row tile 0, O outputs via row tile 1; standard [A;B]
        # channel layout lands directly in each output bank.
        nc.tensor.matmul(OE[0:C, :], lhsT=wct2[0:C, :], rhs=DA[0:C, :],
                         start=True, stop=True, tile_position=(0, 0),
                         skip_group_check=True)
        nc.tensor.matmul(OE[C : 2 * C, :], lhsT=wct2[0:C, :], rhs=DB[0:C, :],
                         start=True, stop=True, tile_position=(0, C),
                         skip_group_check=True)
        nc.tensor.matmul(OO[0:C, :], lhsT=wct2[C : 2 * C, :], rhs=DA[C : 2 * C, :],
                         start=True, stop=True, tile_position=(C, 0),
                         skip_group_check=True)
        nc.tensor.matmul(OO[C : 2 * C, :], lhsT=wct2[C : 2 * C, :],
                         rhs=DB[C : 2 * C, :],
                         start=True, stop=True, tile_position=(C, C),
                         skip_group_check=True)
        # bias add into the (128, 2048) staging tile; 1 MiB output DMA / 2 blocks
        q, qi = divmod(k, 2)
        if qi == 0:
            zcur["t"] = o2p.tile([128, 4 * HPX], f32, tag="o2", name=f"zt{k}")
        zt = zcur["t"]
        zb = 2 * qi * HPX
        nc.vector.tensor_scalar_add(zt[:, zb : zb + HPX], OE[:, :], bc2[:, 0:1])
        nc.vector.tensor_scalar_add(
            zt[:, zb + HPX : zb + 2 * HPX], OO[:, :], bc2[:, 0:1]
        )
        if qi == 1:
            nc.sync.dma_start(out_d[:, q * 4 * HPX : (q + 1) * 4 * HPX], zt[:, :])

    pending = None
    zcur = {"t": None}
    for k in range(NBLK):
        st = dw_stage(k)
        if pending is not None:
            conv1x1_stage(*pending)
        pending = st
    conv1x1_stage(*pending)


# ---------------------------------------------------------------------------
# host-side entry point
# ---------------------------------------------------------------------------

_PROGRAM_CACHE: dict[str, bass.Bass] = {}


def _get_program(x_mode: str) -> bass.Bass:
    if x_mode not in _PROGRAM_CACHE:
        _PROGRAM_CACHE[x_mode] = build_program(x_mode)
    return _PROGRAM_CACHE[x_mode]


def _host_prep(inputs: dict, x_mode: str):
    import ml_dtypes

    x = np.asarray(inputs["x"], dtype=np.float32)
    d = np.asarray(inputs["d"], dtype=np.float32)
    Wk1 = np.asarray(inputs["Wk1"], dtype=np.float32)
    Wk2 = np.asarray(inputs["Wk2"], dtype=np.float32)
    Wc = np.asarray(inputs["Wc"], dtype=np.float32)
    bc = np.asarray(inputs["bc"], dtype=np.float32)

    wk1t = np.ascontiguousarray(Wk1.T)
    w = Wk2.reshape(C, KK * KK, C).transpose(2, 1, 0)  # (j, t, c)
    wk2td = np.ascontiguousarray(
        np.concatenate([w, w], axis=2).reshape(C, KK * KK * 2 * C)
    )
    wct = np.ascontiguousarray(Wc.T)
    wct2 = np.ascontiguousarray(np.concatenate([wct, wct], axis=0)).astype(
        ml_dtypes.bfloat16
    )
    bc2 = np.ascontiguousarray(np.concatenate([bc, bc]).reshape(2 * C, 1))

    xcast = x.astype(ml_dtypes.bfloat16)

    in_maps = []
    for i in range(NCORES):
        xs = np.ascontiguousarray(xcast[S * i : S * (i + 1)].reshape(S * C, H * W))
        dT = np.ascontiguousarray(d[S * i : S * (i + 1)].T)
        in_maps.append(
            {
                "x": xs,
                "dT": dT,
                "wk1t": wk1t,
                "wk2td": wk2td,
                "wct2": wct2,
                "bc2": bc2,
            }
        )
    return in_maps


def run_on_hw(inputs: dict, x_mode: str = None, **kwargs):
    """Run the SPMD kernel on 8 NeuronCores; returns (output, BassKernelResults)."""
    from concourse.bass_utils import run_bass_kernel_spmd

    x_mode = x_mode or X_MODE
    nc = _get_program(x_mode)
    in_maps = _host_prep(inputs, x_mode)
    res = run_bass_kernel_spmd(nc, in_maps, core_ids=list(range(NCORES)), **kwargs)
    outs = res.results
    B = S * NCORES
    out = np.empty((B, C, H, W), dtype=np.float32)
    for i in range(NCORES):
        out[S * i : S * (i + 1)] = outs[i]["out"].reshape(S, C, H, W)
    return out, res


def kernel(**inputs) -> np.ndarray:
    out, _ = run_on_hw(inputs)
    return out


if __name__ == "__main__":
    nc = build_program()
    print("program built OK")



# revision 3
# speedup vs baseline: 2.1085x; 2.1085x over previous
"""Trainium2 Bass kernel for nn_DA_conv: per-sample dynamic depthwise 3x3 conv
(+LeakyReLU) followed by a 1x1 pointwise conv, with the 3x3 kernels produced by
a small per-sample MLP.

Strategy (8 NeuronCores, pure batch data-parallel, 2 samples per core):
  - SBUF partition p = (sample s = p//64, channel c = p%64); fp16 feature map
    resident in SBUF with zero-padded borders (row stride 132, halo rows).
  - Kernel-generating MLP on TensorE (tiny f32 matmuls), diag weights built on
    VectorE from an iota identity.
  - Depthwise 3x3 conv is split spatially between engines, 4-row windows
    (512 px = one PSUM bank) as the unit:
      * PE windows: 9 PSUM-accumulating 128-partition block-diagonal fp16
        matmuls (both samples in one matmul -> 1 cycle/row).
      * DVE windows: per-partition MAC chains (scalar_tensor_tensor) into two
        f32 SBUF partial accumulators; tap-0 products on GPSIMD (tensor_scalar
        mul), the A+B merge on GPSIMD (tensor_tensor add). This offloads
        2 of the 9 taps' worth of VectorE time onto otherwise-idle engines.
  - LeakyReLU evacuation (PSUM/SBUF acc -> fp16 D) on ScalarE via Prelu.
  - 1x1 conv = one 128x128 block-diag fp16 matmul per window; bias added
    during the PSUM->SBUF evac on ScalarE via Prelu(alpha=1) with a bias AP.
  - Output staged as fp16 and DMA'd out; host upcasts to f32.
"""

import os
import sys

sys.path.insert(0, "/opt/trn_rl_repo")

from contextlib import ExitStack

import numpy as np

import concourse.bacc as bacc
import concourse.bass as bass
import concourse.mybir as mybir
import concourse.tile as tile

S = 2            # samples per core
C = 64           # channels
H = W = 128      # spatial
KK = 3           # conv kernel size
NCORES = 8
RS = W + 4       # padded row stride (132 fp16 -> 264B; pads at cols 0,129..131)
RP = H + 2       # padded row count (top/bottom halo)
XFREE = RP * RS
WR = 4           # image rows per window
NWIN = H // WR   # 32 windows
WPX = WR * W     # 512 px per window = one PSUM bank of f32

f32 = mybir.dt.float32
fp16 = mybir.dt.float16
i32 = mybir.dt.int32

PRELU = mybir.ActivationFunctionType.Prelu
TAPS = [(di, dj) for di in range(KK) for dj in range(KK)]  # t = di*3 + dj

# Window ownership: PE windows run the dw conv as matmuls; DVE windows as
# vector MAC chains. Assigned in adjacent pairs so PSUM/acc evacuations can be
# batched as [128, 1024] Prelu ops. Interleaved so no engine starves while the
# input DMA is still streaming (windows arrive in order).
N_DVE_PAIRS = int(os.environ.get("DA_NDVE", "5"))
# pair unit u covers windows (2u, 2u+1); spread DVE pairs across the image
_DVE_UNITS_BY_COUNT = {
    0: [],
    1: [2],
    2: [2, 8],
    3: [1, 6, 11],
    4: [1, 5, 9, 13],
    5: [1, 4, 7, 10, 13],
    6: [1, 3, 6, 9, 11, 14],
    7: [1, 3, 5, 7, 9, 11, 13],
    8: [0, 2, 4, 6, 8, 10, 12, 14],
}
DVE_UNITS = _DVE_UNITS_BY_COUNT[N_DVE_PAIRS]


def build_program() -> bass.Bass:
    nc = bacc.Bacc("TRN2", target_bir_lowering=False, debug=False)

    x_d = nc.dram_tensor("x", [S * C, H * W], fp16, kind="ExternalInput").ap()
    # packed f32 weights: [wk1t (64) | wk2td (1152) | bc2 (1) | dT (2)]
    pk_d = nc.dram_tensor("packed", [S * C, 1219], f32, kind="ExternalInput").ap()
    # block-diagonal duplicated 1x1 weights, fp16
    wcb_d = nc.dram_tensor("wcb", [S * C, S * C], fp16, kind="ExternalInput").ap()
    out_d = nc.dram_tensor("out", [S * C, H * W], fp16, kind="ExternalOutput").ap()

    with tile.TileContext(nc) as tc, ExitStack() as ctx:
        _body(ctx, tc, x_d, pk_d, wcb_d, out_d)
    nc.compile()
    return nc


def _body(ctx, tc, x_d, pk_d, wcb_d, out_d):
    nc = tc.nc
    const = ctx.enter_context(tc.tile_pool(name="const", bufs=1))
    xpool = ctx.enter_context(tc.tile_pool(name="xs", bufs=1))
    dpool = ctx.enter_context(tc.tile_pool(name="dd", bufs=1))
    pdw = ctx.enter_context(tc.tile_pool(name="pdw", bufs=2, space="PSUM"))
    po = ctx.enter_context(tc.tile_pool(name="po", bufs=2, space="PSUM"))

    # ---------------- weight loads ----------------
    pk = const.tile([2 * C, 1219], f32)
    nc.sync.dma_start(pk[:, :], pk_d)
    wcb = const.tile([2 * C, 2 * C], fp16)
    nc.sync.dma_start(wcb[:, :], wcb_d)

    wk1t = pk[0:C, 0:C]
    wk2td = pk[0:C, C : C + KK * KK * 2 * C]
    bc2 = pk[:, 1216:1217]
    dts = pk[0:C, 1217:1219]

    # ---------------- resident padded feature map ----------------
    xs = xpool.tile([2 * C, XFREE], fp16)
    # top halo row + row-1 left pad, bottom halo row; then the interior pad
    # columns: right pads (129..131) of row r are contiguous with the left pad
    # (col 0) of row r+1, so one strided memset covers all of them.
    nc.vector.memset(xs[:, 0 : RS + 1], 0.0)
    nc.vector.memset(xs[:, (RP - 1) * RS : RP * RS], 0.0)
    pads = xs[:, W + 1 : W + 1 + (H + 1) * RS].rearrange("p (r w) -> p r w", w=RS)
    nc.vector.memset(pads[:, :, 0:4], 0.0)

    # x DMA in chunks; first chunks small so compute can start early
    chunk_rows = [8, 8, 16, 16, 16, 16, 16, 16, 16]
    r0 = 0
    for cr in chunk_rows:
        src = x_d[:, r0 * W : (r0 + cr) * W].rearrange("p (r w) -> p r w", w=W)
        o = (r0 + 1) * RS + 1
        dst = xs[:, o : o + cr * RS].rearrange("p (r w) -> p r w", w=RS)[:, :, 0:W]
        nc.sync.dma_start(dst, src)
        r0 += cr

    # ---------------- kernel-generating MLP (f32, exact) ----------------
    hid_ps = po.tile([C, S], f32, tag="po")
    nc.tensor.matmul(hid_ps[:, :], lhsT=wk1t, rhs=dts, start=True, stop=True)
    hid_sb = const.tile([C, S], f32)
    nc.scalar.activation(hid_sb[:, :], hid_ps[:, :], PRELU, alpha=0.1)

    # kern tap columns: kcols[s*64+c, t] = kern[s, c*9+t]
    kcols = const.tile([2 * C, KK * KK], f32)
    for t in range(KK * KK):
        kp = po.tile([2 * C, S], f32, tag="po")
        nc.tensor.matmul(
            kp[:, :],
            lhsT=wk2td[:, t * 128 : (t + 1) * 128],
            rhs=hid_sb[:, :],
            start=True, stop=True,
        )
        nc.vector.tensor_copy(kcols[0:C, t : t + 1], kp[0:C, 0:1])
        nc.vector.tensor_copy(kcols[C : 2 * C, t : t + 1], kp[C : 2 * C, 1:2])

    # identity -> per-tap 128x128 block-diagonal fp16 weights
    id_i = const.tile([128, 128], i32)
    nc.gpsimd.iota(id_i[:, :], pattern=[[1, 128]], base=0, channel_multiplier=-1)
    idf = const.tile([128, 128], f32)
    nc.vector.tensor_scalar(idf[:, :], id_i[:, :], 0, None, mybir.AluOpType.is_equal)
    diag = const.tile([128, KK * KK * 128], fp16)
    for t in range(KK * KK):
        nc.vector.tensor_scalar_mul(
            diag[:, t * 128 : (t + 1) * 128], idf[:, :], kcols[:, t : t + 1]
        )

    # ---------------- main loop ----------------
    xrows = xs[:, :].rearrange("p (r w) -> p r w", w=RS)
    D = dpool.tile([2 * C, H * W], fp16)    # lrelu(dw), 1x1 rhs
    D2 = dpool.tile([2 * C, H * W], fp16)   # 1x1 out + bias, DMA staging
    accv = dpool.tile([2 * C, 2 * N_DVE_PAIRS * WPX], f32, name="accv") if N_DVE_PAIRS else None
    accB = dpool.tile([2 * C, WPX], f32, tag="accB", name="accB") if N_DVE_PAIRS else None

    def win_ap(w, di, dj):
        # rhs/in0 window for tap (di, dj) over image rows 4w..4w+3
        return xrows[:, WR * w + di : WR * w + di + WR, dj : dj + W]

    def dw_pe_pair(u):
        # both windows of unit u on the TensorEngine, PSUM-accumulating
        P2 = pdw.tile([128, 2 * WPX], f32, tag="pdw")
        for half in range(2):
            w = 2 * u + half
            dst = P2[:, half * WPX : (half + 1) * WPX]
            for t, (di, dj) in enumerate(TAPS):
                nc.tensor.matmul(
                    dst,
                    lhsT=diag[:, t * 128 : (t + 1) * 128],
                    rhs=win_ap(w, di, dj),
                    start=(t == 0), stop=(t == KK * KK - 1),
                    skip_group_check=True,
                )
        nc.scalar.activation(
            D[:, 2 * u * WPX : (2 * u + 2) * WPX], P2[:, :], PRELU, alpha=0.1
        )

    def dw_dve_pair(u, ui):
        # both windows of unit u as vector MAC chains, two partial
        # accumulators merged on GPSIMD; tap-0 products on GPSIMD
        base = 2 * ui * WPX
        for half in range(2):
            w = 2 * u + half
            a = accv[:, base + half * WPX : base + (half + 1) * WPX]
            b = accB[:, :]
            # accA: taps 0..3 (product by GPSIMD, 3 MACs on DVE)
            nc.gpsimd.tensor_scalar_mul(a, win_ap(w, 0, 0), kcols[:, 0:1])
            for t in range(1, 4):
                nc.vector.scalar_tensor_tensor(
                    a, win_ap(w, *TAPS[t]), kcols[:, t : t + 1], a,
                    op0=mybir.AluOpType.mult, op1=mybir.AluOpType.add,
                )
            # accB: taps 4..8 (product by ScalarE, 4 MACs on DVE)
            nc.scalar.mul(b, win_ap(w, *TAPS[4]), kcols[:, 4:5])
            for t in range(5, 9):
                nc.vector.scalar_tensor_tensor(
                    b, win_ap(w, *TAPS[t]), kcols[:, t : t + 1], b,
                    op0=mybir.AluOpType.mult, op1=mybir.AluOpType.add,
                )
            # merge on GPSIMD
            nc.gpsimd.tensor_tensor(a, a, b, mybir.AluOpType.add)
        nc.scalar.activation(
            D[:, 2 * u * WPX : (2 * u + 2) * WPX],
            accv[:, base : base + 2 * WPX], PRELU, alpha=0.1,
        )

    def conv1x1_pair(u):
        O2 = po.tile([128, 2 * WPX], f32, tag="po")
        for half in range(2):
            w = 2 * u + half
            nc.tensor.matmul(
                O2[:, half * WPX : (half + 1) * WPX],
                lhsT=wcb[:, :],
                rhs=D[:, w * WPX : (w + 1) * WPX],
                start=True, stop=True,
                skip_group_check=True,
            )
        nc.scalar.activation(
            D2[:, 2 * u * WPX : (2 * u + 2) * WPX], O2[:, :], PRELU,
            bias=bc2, scale=1.0, alpha=1.0,
        )

    dve_ui = {u: i for i, u in enumerate(DVE_UNITS)}
    for u in range(NWIN // 2):
        if u in dve_ui:
            dw_dve_pair(u, dve_ui[u])
        else:
            dw_pe_pair(u)
        conv1x1_pair(u)
        if u % 2 == 1:
            o = (u - 1) * 2 * WPX
            nc.sync.dma_start(
                out_d[:, o : o + 4 * WPX], D2[:, o : o + 4 * WPX]
            )


# ---------------------------------------------------------------------------
# host-side entry point
# ---------------------------------------------------------------------------

_PROGRAM_CACHE: dict[str, bass.Bass] = {}


def _get_program() -> bass.Bass:
    if "p" not in _PROGRAM_CACHE:
        _PROGRAM_CACHE["p"] = build_program()
    return _PROGRAM_CACHE["p"]


def _host_prep(inputs: dict):
    x = np.asarray(inputs["x"], dtype=np.float32)
    d = np.asarray(inputs["d"], dtype=np.float32)
    Wk1 = np.asarray(inputs["Wk1"], dtype=np.float32)
    Wk2 = np.asarray(inputs["Wk2"], dtype=np.float32)
    Wc = np.asarray(inputs["Wc"], dtype=np.float32)
    bc = np.asarray(inputs["bc"], dtype=np.float32)

    wk1t = np.ascontiguousarray(Wk1.T)                      # (64, 64)
    w = Wk2.reshape(C, KK * KK, C).transpose(2, 1, 0)       # (j, t, c)
    wk2td = np.concatenate([w, w], axis=2).reshape(C, KK * KK * 2 * C)
    bc2 = np.concatenate([bc, bc]).reshape(2 * C, 1)

    wcb = np.zeros((2 * C, 2 * C), dtype=np.float32)
    wcb[0:C, 0:C] = Wc.T
    wcb[C:, C:] = Wc.T
    wcb = wcb.astype(np.float16)

    xcast = x.astype(np.float16)

    in_maps = []
    for i in range(NCORES):
        xi = np.ascontiguousarray(xcast[S * i : S * (i + 1)].reshape(S * C, H * W))
        dT = d[S * i : S * (i + 1)].T                       # (64, 2)
        packed = np.zeros((2 * C, 1219), dtype=np.float32)
        packed[0:C, 0:C] = wk1t
        packed[0:C, C : C + KK * KK * 2 * C] = wk2td
        packed[:, 1216:1217] = bc2
        packed[0:C, 1217:1219] = dT
        in_maps.append({"x": xi, "packed": packed, "wcb": wcb})
    return in_maps


def run_on_hw(inputs: dict, **kwargs):
    """Run the SPMD kernel on 8 NeuronCores; returns (output, BassKernelResults)."""
    from concourse.bass_utils import run_bass_kernel_spmd

    nc = _get_program()
    in_maps = _host_prep(inputs)
    res = run_bass_kernel_spmd(nc, in_maps, core_ids=list(range(NCORES)), **kwargs)
    outs = res.results
    B = S * NCORES
    out = np.empty((B, C, H, W), dtype=np.float32)
    for i in range(NCORES):
        out[S * i : S * (i + 1)] = (
            outs[i]["out"].view(np.float16).astype(np.float32).reshape(S, C, H, W)
        )
    return out, res


def kernel(**inputs) -> np.ndarray:
    out, _ = run_on_hw(inputs)
    return out


if __name__ == "__main__":
    nc = build_program()
    print("program built OK")


# revision 5
# speedup vs baseline: 2.1606x; 1.0247x over previous
"""Trainium2 Bass kernel for nn_DA_conv: per-sample dynamic depthwise 3x3 conv
(+LeakyReLU) followed by a 1x1 pointwise conv, with the 3x3 kernels produced by
a small per-sample MLP.

Strategy (8 NeuronCores, pure batch data-parallel, 2 samples per core):
  - SBUF partition p = (sample s = p//64, channel c = p%64); fp16 feature map
    resident in SBUF with zero-padded borders (row stride 132, halo rows).
  - Kernel-generating MLP on TensorE (tiny f32 matmuls), diag weights built on
    VectorE from an iota identity.
  - Depthwise 3x3 conv is split spatially between engines, 4-row windows
    (512 px = one PSUM bank) as the unit:
      * PE windows: 9 PSUM-accumulating 128-partition block-diagonal fp16
        matmuls (both samples in one matmul -> 1 cycle/row).
      * DVE windows: tap-0 product on GPSIMD (tensor_scalar mul into an f32
        SBUF accumulator), then 8 in-place scalar_tensor_tensor MACs on
        VectorE. DVE units sit early in the image so the trailing 1x1 matmuls
        never wait on VectorE.
  - LeakyReLU evacuation (PSUM / SBUF acc -> fp16 D) on ScalarE via Prelu.
  - 1x1 conv = one 128x128 block-diag fp16 matmul per window; bias added
    during the PSUM->SBUF evac on ScalarE via Prelu(alpha=1) with a bias AP.
  - Output staged as fp16 and DMA'd out; host upcasts to f32.
  - First/last units run as single windows to shorten the startup and
    drain chains; weights arrive as a small DMA (MLP inputs) + a second DMA
    (Wk2) so the kernel-MLP overlaps the first x chunks.
"""

import os
import sys

sys.path.insert(0, "/opt/trn_rl_repo")

from contextlib import ExitStack

import numpy as np

import concourse.bacc as bacc
import concourse.bass as bass
import concourse.mybir as mybir
import concourse.tile as tile

S = 2            # samples per core
C = 64           # channels
H = W = 128      # spatial
KK = 3           # conv kernel size
NCORES = 8
RS = W + 4       # padded row stride (132 fp16 -> 264B; pads at cols 0,129..131)
RP = H + 2       # padded row count (top/bottom halo)
XFREE = RP * RS
WR = 4           # image rows per window
NWIN = H // WR   # 32 windows
NU = NWIN // 2   # 16 pair units
WPX = WR * W     # 512 px per window = one PSUM bank of f32

f32 = mybir.dt.float32
fp16 = mybir.dt.float16
i32 = mybir.dt.int32

PRELU = mybir.ActivationFunctionType.Prelu
TAPS = [(di, dj) for di in range(KK) for dj in range(KK)]  # t = di*3 + dj

# Pair units owned by the DVE MAC path (early-middle of the image: late
# windows stay on PE so the tail never waits on VectorE).
N_DVE_UNITS = int(os.environ.get("DA_NDVE", "5"))
DVE_UNITS = [1, 3, 5, 7, 9, 11][:N_DVE_UNITS]


def build_program() -> bass.Bass:
    nc = bacc.Bacc("TRN2", target_bir_lowering=False, debug=False)

    x_d = nc.dram_tensor("x", [S * C, H * W], fp16, kind="ExternalInput").ap()
    # small weights: [wk1t (64) | bc2 (1) | dT (2)]
    pk_d = nc.dram_tensor("packed", [S * C, C + 3], f32, kind="ExternalInput").ap()
    wk2_d = nc.dram_tensor("wk2td", [C, KK * KK * 2 * C], f32,
                           kind="ExternalInput").ap()
    # block-diagonal duplicated 1x1 weights, fp16
    wcb_d = nc.dram_tensor("wcb", [S * C, S * C], fp16, kind="ExternalInput").ap()
    out_d = nc.dram_tensor("out", [S * C, H * W], fp16, kind="ExternalOutput").ap()

    with tile.TileContext(nc) as tc, ExitStack() as ctx:
        _body(ctx, tc, x_d, pk_d, wk2_d, wcb_d, out_d)
    nc.compile()
    return nc


def _body(ctx, tc, x_d, pk_d, wk2_d, wcb_d, out_d):
    nc = tc.nc
    const = ctx.enter_context(tc.tile_pool(name="const", bufs=1))
    xpool = ctx.enter_context(tc.tile_pool(name="xs", bufs=1))
    dpool = ctx.enter_context(tc.tile_pool(name="dd", bufs=1))
    pdw = ctx.enter_context(tc.tile_pool(name="pdw", bufs=2, space="PSUM"))
    po = ctx.enter_context(tc.tile_pool(name="po", bufs=2, space="PSUM"))

    # ---------------- weight loads ----------------
    pk = const.tile([2 * C, C + 3], f32)
    nc.sync.dma_start(pk[:, :], pk_d)
    wk2td = const.tile([C, KK * KK * 2 * C], f32)
    nc.sync.dma_start(wk2td[:, :], wk2_d)

    wk1t = pk[0:C, 0:C]
    bc2 = pk[:, C : C + 1]
    dts = pk[0:C, C + 1 : C + 3]

    # ---------------- resident padded feature map ----------------
    xs = xpool.tile([2 * C, XFREE], fp16)
    # top halo row + row-1 left pad, bottom halo row; then the interior pad
    # columns: right pads (129..131) of row r are contiguous with the left pad
    # (col 0) of row r+1, so one strided memset covers all of them.
    nc.gpsimd.memset(xs[:, 0 : RS + 1], 0.0)
    nc.gpsimd.memset(xs[:, (RP - 1) * RS : RP * RS], 0.0)
    pads = xs[:, W + 1 : W + 1 + (H + 1) * RS].rearrange("p (r w) -> p r w", w=RS)
    nc.gpsimd.memset(pads[:, :, 0:4], 0.0)

    # x DMA in chunks; first chunks small so compute can start early
    def dma_chunk(r0, cr):
        src = x_d[:, r0 * W : (r0 + cr) * W].rearrange("p (r w) -> p r w", w=W)
        o = (r0 + 1) * RS + 1
        dst = xs[:, o : o + cr * RS].rearrange("p (r w) -> p r w", w=RS)[:, :, 0:W]
        nc.sync.dma_start(dst, src)

    dma_chunk(0, 8)
    wcb = const.tile([2 * C, 2 * C], fp16)
    nc.sync.dma_start(wcb[:, :], wcb_d)
    r0 = 8
    for cr in [8, 16, 16, 16, 16, 16, 16, 16]:
        dma_chunk(r0, cr)
        r0 += cr

    # ---------------- kernel-generating MLP (f32, exact) ----------------
    hid_ps = po.tile([C, S], f32, tag="po")
    nc.tensor.matmul(hid_ps[:, :], lhsT=wk1t, rhs=dts, start=True, stop=True)
    hid_sb = const.tile([C, S], f32)
    nc.scalar.activation(hid_sb[:, :], hid_ps[:, :], PRELU, alpha=0.1)

    # kern tap columns: kcols[s*64+c, t] = kern[s, c*9+t]
    kcols = const.tile([2 * C, KK * KK], f32)
    for t in range(KK * KK):
        kp = po.tile([2 * C, S], f32, tag="po")
        nc.tensor.matmul(
            kp[:, :],
            lhsT=wk2td[:, t * 128 : (t + 1) * 128],
            rhs=hid_sb[:, :],
            start=True, stop=True,
        )
        nc.vector.tensor_copy(kcols[0:C, t : t + 1], kp[0:C, 0:1])
        nc.vector.tensor_copy(kcols[C : 2 * C, t : t + 1], kp[C : 2 * C, 1:2])

    # identity -> per-tap 128x128 block-diagonal fp16 weights
    id_i = const.tile([128, 128], i32)
    nc.gpsimd.iota(id_i[:, :], pattern=[[1, 128]], base=0, channel_multiplier=-1)
    idf = const.tile([128, 128], f32)
    nc.vector.tensor_scalar(idf[:, :], id_i[:, :], 0, None, mybir.AluOpType.is_equal)
    diag = const.tile([128, KK * KK * 128], fp16)
    for t in range(KK * KK):
        nc.vector.tensor_scalar_mul(
            diag[:, t * 128 : (t + 1) * 128], idf[:, :], kcols[:, t : t + 1]
        )

    # ---------------- main loop ----------------
    xrows = xs[:, :].rearrange("p (r w) -> p r w", w=RS)
    D = dpool.tile([2 * C, H * W], fp16)    # lrelu(dw), 1x1 rhs
    D2 = dpool.tile([2 * C, H * W], fp16)   # 1x1 out + bias, DMA staging
    n_dve_w = 2 * len(DVE_UNITS)
    accv = (dpool.tile([2 * C, n_dve_w * WPX], f32, name="accv")
            if n_dve_w else None)
    dve_w_idx = {}
    for i, u in enumerate(DVE_UNITS):
        dve_w_idx[2 * u] = 2 * i
        dve_w_idx[2 * u + 1] = 2 * i + 1

    def win_ap(w, di, dj):
        # rhs/in0 window for tap (di, dj) over image rows 4w..4w+3
        return xrows[:, WR * w + di : WR * w + di + WR, dj : dj + W]

    def dw_pe(w, dst):
        for t, (di, dj) in enumerate(TAPS):
            nc.tensor.matmul(
                dst,
                lhsT=diag[:, t * 128 : (t + 1) * 128],
                rhs=win_ap(w, di, dj),
                start=(t == 0), stop=(t == KK * KK - 1),
                skip_group_check=True,
            )

    def dw_pe_pair(u):
        P2 = pdw.tile([128, 2 * WPX], f32, tag="pdw")
        for half in range(2):
            dw_pe(2 * u + half, P2[:, half * WPX : (half + 1) * WPX])
        nc.scalar.activation(
            D[:, 2 * u * WPX : (2 * u + 2) * WPX], P2[:, :], PRELU, alpha=0.1
        )

    def dw_pe_single(w):
        P1 = pdw.tile([128, WPX], f32, tag="pdw", name=f"ps{w}")
        dw_pe(w, P1[:, :])
        nc.scalar.activation(
            D[:, w * WPX : (w + 1) * WPX], P1[:, :], PRELU, alpha=0.1
        )

    def dw_dve_pair(u):
        # tap-0 product on GPSIMD, taps 1..8 as in-place DVE MAC chain
        for half in range(2):
            w = 2 * u + half
            a = accv[:, dve_w_idx[w] * WPX : (dve_w_idx[w] + 1) * WPX]
            nc.gpsimd.tensor_scalar_mul(a, win_ap(w, 0, 0), kcols[:, 0:1])
            for t in range(1, KK * KK):
                nc.vector.scalar_tensor_tensor(
                    a, win_ap(w, *TAPS[t]), kcols[:, t : t + 1], a,
                    op0=mybir.AluOpType.mult, op1=mybir.AluOpType.add,
                )
        base = dve_w_idx[2 * u] * WPX
        nc.scalar.activation(
            D[:, 2 * u * WPX : (2 * u + 2) * WPX],
            accv[:, base : base + 2 * WPX], PRELU, alpha=0.1,
        )

    def conv1x1_pair(u):
        O2 = po.tile([128, 2 * WPX], f32, tag="po")
        for half in range(2):
            w = 2 * u + half
            nc.tensor.matmul(
                O2[:, half * WPX : (half + 1) * WPX],
                lhsT=wcb[:, :],
                rhs=D[:, w * WPX : (w + 1) * WPX],
                start=True, stop=True,
                skip_group_check=True,
            )
        nc.scalar.activation(
            D2[:, 2 * u * WPX : (2 * u + 2) * WPX], O2[:, :], PRELU,
            bias=bc2, scale=1.0, alpha=1.0,
        )

    def conv1x1_single(w):
        O1 = po.tile([128, WPX], f32, tag="po", name=f"po{w}")
        nc.tensor.matmul(
            O1[:, :], lhsT=wcb[:, :], rhs=D[:, w * WPX : (w + 1) * WPX],
            start=True, stop=True, skip_group_check=True,
        )
        nc.scalar.activation(
            D2[:, w * WPX : (w + 1) * WPX], O1[:, :], PRELU,
            bias=bc2, scale=1.0, alpha=1.0,
        )

    for u in range(NU):
        if u == 0:
            dw_pe_single(0)
            dw_pe_single(1)
        elif u == NU - 1:
            dw_pe_single(2 * u)
            dw_pe_single(2 * u + 1)
        elif 2 * u in dve_w_idx:
            dw_dve_pair(u)
        else:
            dw_pe_pair(u)
        if u == NU - 1:
            conv1x1_single(2 * u)
            conv1x1_single(2 * u + 1)
            nc.sync.dma_start(
                out_d[:, 2 * u * WPX : (2 * u + 1) * WPX],
                D2[:, 2 * u * WPX : (2 * u + 1) * WPX],
            )
            nc.sync.dma_start(
                out_d[:, (2 * u + 1) * WPX : (2 * u + 2) * WPX],
                D2[:, (2 * u + 1) * WPX : (2 * u + 2) * WPX],
            )
        else:
            conv1x1_pair(u)
            if u % 2 == 1:
                o = (u - 1) * 2 * WPX
                nc.sync.dma_start(out_d[:, o : o + 4 * WPX], D2[:, o : o + 4 * WPX])
    # windows 28..29 (unit 14) flushed here (unit 15 handled its own)
    o = 14 * 2 * WPX
    nc.sync.dma_start(out_d[:, o : o + 2 * WPX], D2[:, o : o + 2 * WPX])


# ---------------------------------------------------------------------------
# host-side entry point
# ---------------------------------------------------------------------------

_PROGRAM_CACHE: dict[str, bass.Bass] = {}


def _get_program() -> bass.Bass:
    if "p" not in _PROGRAM_CACHE:
        _PROGRAM_CACHE["p"] = build_program()
    return _PROGRAM_CACHE["p"]


def _host_prep(inputs: dict):
    x = np.asarray(inputs["x"], dtype=np.float32)
    d = np.asarray(inputs["d"], dtype=np.float32)
    Wk1 = np.asarray(inputs["Wk1"], dtype=np.float32)
    Wk2 = np.asarray(inputs["Wk2"], dtype=np.float32)
    Wc = np.asarray(inputs["Wc"], dtype=np.float32)
    bc = np.asarray(inputs["bc"], dtype=np.float32)

    wk1t = np.ascontiguousarray(Wk1.T)                      # (64, 64)
    w = Wk2.reshape(C, KK * KK, C).transpose(2, 1, 0)       # (j, t, c)
    wk2td = np.ascontiguousarray(
        np.concatenate([w, w], axis=2).reshape(C, KK * KK * 2 * C)
    )
    bc2 = np.concatenate([bc, bc]).reshape(2 * C, 1)

    wcb = np.zeros((2 * C, 2 * C), dtype=np.float32)
    wcb[0:C, 0:C] = Wc.T
    wcb[C:, C:] = Wc.T
    wcb = wcb.astype(np.float16)

    xcast = x.astype(np.float16)

    in_maps = []
    for i in range(NCORES):
        xi = np.ascontiguousarray(xcast[S * i : S * (i + 1)].reshape(S * C, H * W))
        dT = d[S * i : S * (i + 1)].T                       # (64, 2)
        packed = np.zeros((2 * C, C + 3), dtype=np.float32)
        packed[0:C, 0:C] = wk1t
        packed[:, C : C + 1] = bc2
        packed[0:C, C + 1 : C + 3] = dT
        in_maps.append(
            {"x": xi, "packed": packed, "wk2td": wk2td, "wcb": wcb}
        )
    return in_maps


def run_on_hw(inputs: dict, **kwargs):
    """Run the SPMD kernel on 8 NeuronCores; returns (output, BassKernelResults)."""
    from concourse.bass_utils import run_bass_kernel_spmd

    nc = _get_program()
    in_maps = _host_prep(inputs)
    res = run_bass_kernel_spmd(nc, in_maps, core_ids=list(range(NCORES)), **kwargs)
    outs = res.results
    B = S * NCORES
    out = np.empty((B, C, H, W), dtype=np.float32)
    for i in range(NCORES):
        out[S * i : S * (i + 1)] = (
            outs[i]["out"].view(np.float16).astype(np.float32).reshape(S, C, H, W)
        )
    return out, res


def kernel(**inputs) -> np.ndarray:
    out, _ = run_on_hw(inputs)
    return out


if __name__ == "__main__":
    nc = build_program()
    print("program built OK")
